# revision 7
# baseline (speedup 1.0000x reference)
"""Trainium2 Bass kernel: batched causal single-head self-attention.

Reference computation (per batch b):
    q = x @ Wq; k = x @ Wk; v = x @ Wv          # [T, H] each, contraction over E
    S = (q @ k^T) / sqrt(H)                     # [T, T]
    P = softmax(causal_mask(S), axis=-1)
    out = P @ v                                 # [T, H]

Shapes: x [512, 256, 384] f32, W* [384, 64] f32, out [512, 256, 64] f32.
Sharding: pure data parallel, 64 batches per NeuronCore across 8 cores.

Device algorithm per batch (all matmul operands bf16, fp32 PSUM accumulation):
  - host ships xT = x^T per batch ([E, T] layout, E on partitions) so every
    matmul has its contraction dim on partitions.
  - qk^T = [Wq|Wk]^T @ xT      (one packed 128-wide stationary, 3 E-chunks)
  - v^T  = Wv^T @ xT           (3 E-chunks)
  - v    = PE-transpose of v^T, with a ones row appended so the transposed
    tile carries a ones column (used to get softmax denominators for free)
  - S^T  = k^T.T @ q^T         ([tk, tq] layout; lower-left T/4 block skipped)
  - P    = exp(0.125 * S^T)    (ScalarE; no max-subtraction needed, |s|<~45)
  - P   *= causal 0/1 mask     (multiplicative, on the two diagonal blocks)
  - outA = P^T-contracted V: out_aug[tq, 0:65] = sum_tk P[tk,tq] * [v|1][tk]
    so col 64 = softmax denominator; divide + store.
"""

import numpy as np
import ml_dtypes

B, T, E, H = 512, 256, 384, 64
NCORES = 8
BPC = B // NCORES  # 64
P = 128
EC = E // P  # 3
HP1 = H + 1  # 65

_cache: dict = {}


def _install_ntff_hook():
    """Shim antenv.axon_hooks (absent in this image) so run_bass_kernel_spmd
    trace=True can capture NTFF profiles via the axon .so's C ABI."""
    import contextlib
    import ctypes
    import sys
    import types

    if "antenv.axon_hooks" in sys.modules:
        return
    so_path = "/opt/axon/libaxon_pjrt.so"
    lib = ctypes.CDLL(so_path)
    if not hasattr(lib, "axon_start_nrt_profile"):
        return
    lib.axon_start_nrt_profile.argtypes = [
        ctypes.POINTER(ctypes.c_int64),
        ctypes.c_size_t,
    ]
    lib.axon_start_nrt_profile.restype = ctypes.c_int64
    lib.axon_stop_nrt_profile.argtypes = [ctypes.c_char_p]
    lib.axon_stop_nrt_profile.restype = ctypes.c_int64

    @contextlib.contextmanager
    def _hook(output_dir, device_ids):
        import jax

        jax.devices()
        if device_ids:
            ids = (ctypes.c_int64 * len(device_ids))(*device_ids)
            rc = lib.axon_start_nrt_profile(ids, len(device_ids))
        else:
            rc = lib.axon_start_nrt_profile(None, 0)
        if rc != 0:
            raise RuntimeError(f"axon_start_nrt_profile rc={rc}")
        try:
            yield
        finally:
            n = lib.axon_stop_nrt_profile(str(output_dir).encode())
            if n < 0:
                raise RuntimeError(f"axon_stop_nrt_profile rc={n}")
            print(f"profile: {n} file(s) written to {output_dir}", file=sys.stderr)

    mod = types.ModuleType("antenv.axon_hooks")
    _state = {"hook": _hook}
    mod.get_axon_ntff_profile_hook = lambda: _state["hook"]
    mod.set_axon_ntff_profile_hook = lambda h: _state.__setitem__("hook", h)
    sys.modules["antenv.axon_hooks"] = mod


def _build_program(bpc):
    import concourse.bacc as bacc
    import concourse.mybir as mybir
    import concourse.tile as tile

    f32 = mybir.dt.float32
    bf16 = mybir.dt.bfloat16
    Exp = mybir.ActivationFunctionType.Exp
    Mult = mybir.AluOpType.mult

    nc = bacc.Bacc(
        "TRN2",
        target_bir_lowering=False,
        debug=False,
        enable_asserts=False,
        num_devices=NCORES,
    )
    xt_d = nc.dram_tensor("xt", [bpc, P, EC, T], bf16, kind="ExternalInput").ap()
    wqk_d = nc.dram_tensor("wqk", [P, EC, P], bf16, kind="ExternalInput").ap()
    wv_d = nc.dram_tensor("wv", [P, EC, H], bf16, kind="ExternalInput").ap()
    # ones-padded multiplicative causal mask for P^T tiles:
    # cols 0:128 = tril01 (tk<=tq), cols 128:256 = 1, cols 256:384 = tril01
    um_d = nc.dram_tensor("um", [P, 3 * P], bf16, kind="ExternalInput").ap()
    iden_d = nc.dram_tensor("iden", [HP1, HP1], f32, kind="ExternalInput").ap()
    out_d = nc.dram_tensor("out", [bpc, T, H], f32, kind="ExternalOutput").ap()

    # DMA instructions have a flat ~600ns issue cost on the Sync sequencer, so
    # in/out traffic is batched in groups of Q=4 batches; compute + PSUM ops
    # stay pair-granular.
    Q = 4
    assert bpc % Q == 0
    nquads = bpc // Q

    with tile.TileContext(nc) as tc:
        with (
            tc.tile_pool(name="const", bufs=1) as constp,
            tc.tile_pool(name="xin", bufs=4) as xpool,
            tc.tile_pool(name="qksb", bufs=3) as qkpool,
            tc.tile_pool(name="ksh", bufs=3) as kpool,
            tc.tile_pool(name="psb", bufs=3) as ppool,
            tc.tile_pool(name="vaug", bufs=3) as vpool,
            tc.tile_pool(name="osb", bufs=2) as opool,
            tc.tile_pool(name="rec", bufs=2) as rpool,
            tc.tile_pool(name="ps_qk", bufs=2, space="PSUM") as ps_qk,
            tc.tile_pool(name="ps_vt", bufs=2, space="PSUM") as ps_vt,
            tc.tile_pool(name="ps_s", bufs=2, space="PSUM") as ps_s,
            tc.tile_pool(name="ps_tr", bufs=1, space="PSUM") as ps_tr,
            tc.tile_pool(name="ps_o", bufs=1, space="PSUM") as ps_o,
        ):
            wqk = constp.tile([P, EC, P], bf16)
            nc.sync.dma_start(wqk, wqk_d)
            wv = constp.tile([P, EC, H], bf16)
            nc.sync.dma_start(wv, wv_d)
            um = constp.tile([P, 3 * P], bf16)
            nc.sync.dma_start(um, um_d)
            iden = constp.tile([HP1, HP1], f32)
            nc.sync.dma_start(iden, iden_d)
            # v^T staging with a persistent ones row at partition 64 (manual
            # double-buffer so the ones row survives across iterations)
            vtabs = []
            for i in range(2):
                vt = constp.tile([HP1, 2, T], f32, name=f"vtab{i}")
                nc.vector.memset(vt[H : H + 1, :, :], 1.0)
                vtabs.append(vt)

            for qd in range(nquads):
                b0 = Q * qd
                xt = xpool.tile([P, Q, EC, T], bf16)
                nc.sync.dma_start(
                    xt, xt_d[b0 : b0 + Q].rearrange("s p c t -> p s c t")
                )
                qk_sb = qkpool.tile([P, Q, T], bf16)
                k_sb = kpool.tile([H, Q, T], bf16)
                o_sb = opool.tile([P, Q, 2, H], f32)

                v_augs = []
                for prl in range(Q // 2):
                    s0 = 2 * prl
                    pr = qd * (Q // 2) + prl

                    qk_ps = ps_qk.tile([P, 2, T], f32)
                    vt_ps = ps_vt.tile([H, 2, T], f32)
                    for s in range(2):
                        for c in range(EC):
                            nc.tensor.matmul(
                                qk_ps[:, s, :],
                                wqk[:, c, :],
                                xt[:, s0 + s, c, :],
                                start=(c == 0),
                                stop=(c == EC - 1),
                            )
                    for s in range(2):
                        for c in range(EC):
                            nc.tensor.matmul(
                                vt_ps[:, s, :],
                                wv[:, c, :],
                                xt[:, s0 + s, c, :],
                                start=(c == 0),
                                stop=(c == EC - 1),
                            )

                    # [q^T; k^T] PSUM -> SBUF bf16 (ScalarE, exp-set Copy)
                    nc.scalar.copy(qk_sb[:, s0 : s0 + 2, :], qk_ps)

                    vtab = vtabs[pr % 2]
                    nc.vector.tensor_copy(vtab[0:H], vt_ps)

                    tr_ps = ps_tr.tile([P, 2, 2, HP1], f32)
                    for s in range(2):
                        for j in range(2):
                            nc.tensor.transpose(
                                tr_ps[:, s, j, :],
                                vtab[:, s, j * P : (j + 1) * P],
                                iden,
                            )
                    v_aug = vpool.tile([P, 2, 2, HP1], bf16)
                    nc.vector.tensor_copy(v_aug, tr_ps)
                    v_augs.append(v_aug)

                    # k^T partitions 64:128 -> 0:64 (DMA partition shift)
                    nc.sync.dma_start(
                        k_sb[:, s0 : s0 + 2, :], qk_sb[H:P, s0 : s0 + 2, :]
                    )

                for prl in range(Q // 2):
                    s0 = 2 * prl
                    v_aug = v_augs[prl]

                    p_sb = ppool.tile([P, 2, 3 * P], bf16)
                    for s in range(2):
                        s_ps = ps_s.tile([P, 3 * P], f32, name="s_ps")
                        # S^T[tk 0:128, tq 0:256]
                        nc.tensor.matmul(
                            s_ps[:, 0:T],
                            k_sb[:, s0 + s, 0:P],
                            qk_sb[0:H, s0 + s, :],
                            start=True,
                            stop=True,
                        )
                        # S^T[tk 128:256, tq 128:256]
                        nc.tensor.matmul(
                            s_ps[:, T : 3 * P],
                            k_sb[:, s0 + s, P:T],
                            qk_sb[0:H, s0 + s, P:T],
                            start=True,
                            stop=True,
                        )
                        nc.scalar.activation(
                            p_sb[:, s, :], s_ps, Exp, scale=0.125
                        )

                    # multiplicative causal mask on both batches at once
                    nc.vector.tensor_tensor(
                        p_sb,
                        p_sb,
                        um[:, None, :].to_broadcast([P, 2, 3 * P]),
                        Mult,
                    )

                    o_ps = ps_o.tile([P, 2, 2, HP1], f32)
                    for s in range(2):
                        nc.tensor.matmul(
                            o_ps[:, s, 0, :],
                            p_sb[:, s, 0:P],
                            v_aug[:, s, 0, :],
                            start=True,
                            stop=True,
                        )
                        nc.tensor.matmul(
                            o_ps[:, s, 1, :],
                            p_sb[:, s, P:T],
                            v_aug[:, s, 0, :],
                            start=True,
                            stop=False,
                        )
                        nc.tensor.matmul(
                            o_ps[:, s, 1, :],
                            p_sb[:, s, T : 3 * P],
                            v_aug[:, s, 1, :],
                            start=False,
                            stop=True,
                        )

                    rec = rpool.tile([P, 2, 2, 1], f32)
                    nc.vector.reciprocal(rec, o_ps[:, :, :, H : H + 1])
                    nc.vector.tensor_tensor(
                        o_sb[:, s0 : s0 + 2, :, :],
                        o_ps[:, :, :, 0:H],
                        rec.to_broadcast([P, 2, 2, H]),
                        Mult,
                    )

                nc.sync.dma_start(
                    out_d[b0 : b0 + Q].rearrange("s (j p) h -> p s j h", p=P),
                    o_sb,
                )

    nc.compile()
    return nc


def _prep_inputs(x, Wq, Wk, Wv, bpc):
    bf = ml_dtypes.bfloat16
    nb = NCORES * bpc
    x = np.asarray(x, dtype=np.float32)[:nb]
    # [b, t, e] -> [b, p, c, t] with e = c*128 + p
    xt = np.ascontiguousarray(
        x.reshape(nb, T, EC, P).transpose(0, 3, 2, 1)
    ).astype(bf)
    wqk = np.concatenate(
        [np.asarray(Wq, np.float32), np.asarray(Wk, np.float32)], axis=1
    )  # [E, 128]
    wqk = np.ascontiguousarray(wqk.reshape(EC, P, P).transpose(1, 0, 2)).astype(bf)
    wv = np.ascontiguousarray(
        np.asarray(Wv, np.float32).reshape(EC, P, H).transpose(1, 0, 2)
    ).astype(bf)
    tril01 = (np.arange(P)[:, None] <= np.arange(P)[None, :]).astype(np.float32)
    um = np.concatenate([tril01, np.ones((P, P), np.float32), tril01], axis=1).astype(
        bf
    )
    iden = np.eye(HP1, dtype=np.float32)
    per_core = []
    for c in range(NCORES):
        per_core.append(
            {
                "xt": xt[c * bpc : (c + 1) * bpc],
                "wqk": wqk,
                "wv": wv,
                "um": um,
                "iden": iden,
            }
        )
    return per_core


def kernel(x, Wq, Wk, Wv, _trace=False, _bpc=BPC):
    """Full inputs in, full output out. Shards batch dim over 8 NeuronCores."""
    from concourse import bass_utils

    if _trace:
        _install_ntff_hook()

    key = ("prog", _bpc)
    if key not in _cache:
        _cache[key] = _build_program(_bpc)
    nc = _cache[key]

    in_maps = _prep_inputs(x, Wq, Wk, Wv, _bpc)
    res = bass_utils.run_bass_kernel_spmd(
        nc, in_maps, core_ids=list(range(NCORES)), trace=_trace
    )
    _cache["last_result"] = res
    out = np.concatenate([r["out"] for r in res.results], axis=0)
    return out.astype(np.float32)


# revision 9
# speedup vs baseline: 1.1527x; 1.1527x over previous
"""Trainium2 Bass kernel: batched causal single-head self-attention.

Reference computation (per batch b):
    q = x @ Wq; k = x @ Wk; v = x @ Wv          # [T, H] each, contraction over E
    S = (q @ k^T) / sqrt(H)                     # [T, T]
    P = softmax(causal_mask(S), axis=-1)
    out = P @ v                                 # [T, H]

Shapes: x [512, 256, 384] f32, W* [384, 64] f32, out [512, 256, 64] f32.
Sharding: pure data parallel, 64 batches per NeuronCore across 8 cores.

Device algorithm per batch (all matmul operands bf16, fp32 PSUM accumulation):
  - host ships xT = x^T per batch ([E, T] layout, E on partitions) so every
    matmul has its contraction dim on partitions.
  - qk^T = [Wq|Wk]^T @ xT      (one packed 128-wide stationary, 3 E-chunks)
  - v^T  = Wv^T @ xT           (3 E-chunks)
  - v    = PE-transpose of v^T, with a ones row appended so the transposed
    tile carries a ones column (used to get softmax denominators for free)
  - S^T  = k^T.T @ q^T         ([tk, tq] layout; lower-left T/4 block skipped)
  - P    = exp(0.125 * S^T)    (ScalarE; no max-subtraction needed, |s|<~45)
  - P   *= causal 0/1 mask     (multiplicative, on the two diagonal blocks)
  - outA = P^T-contracted V: out_aug[tq, 0:65] = sum_tk P[tk,tq] * [v|1][tk]
    so col 64 = softmax denominator; divide + store.
"""

import numpy as np
import ml_dtypes

B, T, E, H = 512, 256, 384, 64
NCORES = 8
BPC = B // NCORES  # 64
P = 128
EC = E // P  # 3
HP1 = H + 1  # 65

_cache: dict = {}


def _install_ntff_hook():
    """Shim antenv.axon_hooks (absent in this image) so run_bass_kernel_spmd
    trace=True can capture NTFF profiles via the axon .so's C ABI."""
    import contextlib
    import ctypes
    import sys
    import types

    if "antenv.axon_hooks" in sys.modules:
        return
    so_path = "/opt/axon/libaxon_pjrt.so"
    lib = ctypes.CDLL(so_path)
    if not hasattr(lib, "axon_start_nrt_profile"):
        return
    lib.axon_start_nrt_profile.argtypes = [
        ctypes.POINTER(ctypes.c_int64),
        ctypes.c_size_t,
    ]
    lib.axon_start_nrt_profile.restype = ctypes.c_int64
    lib.axon_stop_nrt_profile.argtypes = [ctypes.c_char_p]
    lib.axon_stop_nrt_profile.restype = ctypes.c_int64

    @contextlib.contextmanager
    def _hook(output_dir, device_ids):
        import jax

        jax.devices()
        if device_ids:
            ids = (ctypes.c_int64 * len(device_ids))(*device_ids)
            rc = lib.axon_start_nrt_profile(ids, len(device_ids))
        else:
            rc = lib.axon_start_nrt_profile(None, 0)
        if rc != 0:
            raise RuntimeError(f"axon_start_nrt_profile rc={rc}")
        try:
            yield
        finally:
            n = lib.axon_stop_nrt_profile(str(output_dir).encode())
            if n < 0:
                raise RuntimeError(f"axon_stop_nrt_profile rc={n}")
            print(f"profile: {n} file(s) written to {output_dir}", file=sys.stderr)

    mod = types.ModuleType("antenv.axon_hooks")
    _state = {"hook": _hook}
    mod.get_axon_ntff_profile_hook = lambda: _state["hook"]
    mod.set_axon_ntff_profile_hook = lambda h: _state.__setitem__("hook", h)
    sys.modules["antenv.axon_hooks"] = mod


def _build_program(bpc):
    import concourse.bacc as bacc
    import concourse.mybir as mybir
    import concourse.tile as tile

    f32 = mybir.dt.float32
    bf16 = mybir.dt.bfloat16
    Exp = mybir.ActivationFunctionType.Exp
    Mult = mybir.AluOpType.mult

    nc = bacc.Bacc(
        "TRN2",
        target_bir_lowering=False,
        debug=False,
        enable_asserts=False,
        num_devices=NCORES,
    )
    xt_d = nc.dram_tensor("xt", [bpc, P, EC, T], bf16, kind="ExternalInput").ap()
    wqk_d = nc.dram_tensor("wqk", [P, EC, P], bf16, kind="ExternalInput").ap()
    wv_d = nc.dram_tensor("wv", [P, EC, H], bf16, kind="ExternalInput").ap()
    # ones-padded multiplicative causal mask for P^T tiles:
    # cols 0:128 = tril01 (tk<=tq), cols 128:256 = 1, cols 256:384 = tril01
    um_d = nc.dram_tensor("um", [P, 3 * P], bf16, kind="ExternalInput").ap()
    iden_d = nc.dram_tensor("iden", [HP1, HP1], bf16, kind="ExternalInput").ap()
    out_d = nc.dram_tensor("out", [bpc, T, H], f32, kind="ExternalOutput").ap()

    # DMA instructions have a flat ~600ns issue cost on the Sync sequencer, so
    # in/out traffic is batched in groups of Q=4 batches; compute + PSUM ops
    # stay pair-granular.
    Q = 4
    assert bpc % Q == 0
    nquads = bpc // Q

    with tile.TileContext(nc) as tc:
        with (
            tc.tile_pool(name="const", bufs=1) as constp,
            tc.tile_pool(name="xin", bufs=4) as xpool,
            tc.tile_pool(name="qksb", bufs=3) as qkpool,
            tc.tile_pool(name="ksh", bufs=3) as kpool,
            tc.tile_pool(name="psb", bufs=3) as ppool,
            tc.tile_pool(name="vaug", bufs=3) as vpool,
            tc.tile_pool(name="osb", bufs=2) as opool,
            tc.tile_pool(name="rec", bufs=2) as rpool,
            tc.tile_pool(name="ps_qk", bufs=2, space="PSUM") as ps_qk,
            tc.tile_pool(name="ps_vt", bufs=1, space="PSUM") as ps_vt,
            tc.tile_pool(name="ps_s", bufs=3, space="PSUM") as ps_s,
            tc.tile_pool(name="ps_tr", bufs=1, space="PSUM") as ps_tr,
            tc.tile_pool(name="ps_o", bufs=1, space="PSUM") as ps_o,
        ):
            wqk = constp.tile([P, EC, P], bf16)
            nc.sync.dma_start(wqk, wqk_d)
            wv = constp.tile([P, EC, H], bf16)
            nc.sync.dma_start(wv, wv_d)
            um = constp.tile([P, 3 * P], bf16)
            nc.sync.dma_start(um, um_d)
            iden = constp.tile([HP1, HP1], bf16)
            nc.sync.dma_start(iden, iden_d)
            # v^T staging with a persistent ones row at partition 64 (manual
            # double-buffer so the ones row survives across iterations)
            vtabs = []
            for i in range(2):
                vt = constp.tile([HP1, 2, T], bf16, name=f"vtab{i}")
                nc.vector.memset(vt[H : H + 1, :, :], 1.0)
                vtabs.append(vt)
            # k^T staging padded to 128 partitions with zero rows 64:128 so the
            # scores matmuls use full-width stationaries (FWL) and stream q^T
            # directly from qk_sb (zero k rows null out the garbage rows)
            kabs = []
            for i in range(2):
                kt = constp.tile([P, Q, T], bf16, name=f"kab{i}")
                nc.vector.memset(kt[H:P], 0.0)
                kabs.append(kt)

            for qd in range(nquads):
                b0 = Q * qd
                xt = xpool.tile([P, Q, EC, T], bf16)
                nc.sync.dma_start(
                    xt, xt_d[b0 : b0 + Q].rearrange("s p c t -> p s c t")
                )
                qk_sb = qkpool.tile([P, Q, T], bf16)
                k_sb = kabs[qd % 2]
                o_sb = opool.tile([P, Q, 2, H], f32)

                v_augs = []
                for prl in range(Q // 2):
                    s0 = 2 * prl
                    pr = qd * (Q // 2) + prl

                    qk_ps = ps_qk.tile([P, 2, T], f32)
                    vt_ps = ps_vt.tile([H, 2, T], f32)
                    for s in range(2):
                        for c in range(EC):
                            nc.tensor.matmul(
                                qk_ps[:, s, :],
                                wqk[:, c, :],
                                xt[:, s0 + s, c, :],
                                start=(c == 0),
                                stop=(c == EC - 1),
                            )
                    for s in range(2):
                        for c in range(EC):
                            nc.tensor.matmul(
                                vt_ps[:, s, :],
                                wv[:, c, :],
                                xt[:, s0 + s, c, :],
                                start=(c == 0),
                                stop=(c == EC - 1),
                            )

                    # [q^T; k^T] PSUM -> SBUF bf16 (ScalarE, exp-set Copy)
                    nc.scalar.copy(qk_sb[:, s0 : s0 + 2, :], qk_ps)

                    vtab = vtabs[pr % 2]
                    nc.vector.tensor_copy(vtab[0:H], vt_ps)

                    tr_ps = ps_tr.tile([P, 2, 2, HP1 + 1], bf16)
                    for s in range(2):
                        for j in range(2):
                            nc.tensor.transpose(
                                tr_ps[:, s, j, 0:HP1],
                                vtab[:, s, j * P : (j + 1) * P],
                                iden,
                            )
                    v_aug = vpool.tile([P, 2, 2, HP1], bf16)
                    nc.vector.tensor_copy(v_aug, tr_ps[:, :, :, 0:HP1])
                    v_augs.append(v_aug)

                # k^T partitions 64:128 -> 0:64 (DMA shift), whole quad at once
                nc.sync.dma_start(k_sb[0:H], qk_sb[H:P])

                for prl in range(Q // 2):
                    s0 = 2 * prl
                    v_aug = v_augs[prl]

                    p_sb = ppool.tile([P, 2, 3 * P], bf16)
                    for s in range(2):
                        s_ps = ps_s.tile([P, 3 * P], f32, name="s_ps")
                        # S^T[tk 0:128, tq 0:256]
                        nc.tensor.matmul(
                            s_ps[:, 0:T],
                            k_sb[:, s0 + s, 0:P],
                            qk_sb[:, s0 + s, :],
                            start=True,
                            stop=True,
                        )
                        # S^T[tk 128:256, tq 128:256]
                        nc.tensor.matmul(
                            s_ps[:, T : 3 * P],
                            k_sb[:, s0 + s, P:T],
                            qk_sb[:, s0 + s, P:T],
                            start=True,
                            stop=True,
                        )
                        nc.scalar.activation(
                            p_sb[:, s, :], s_ps, Exp, scale=0.125
                        )

                    # multiplicative causal mask on both batches at once
                    nc.vector.tensor_tensor(
                        p_sb,
                        p_sb,
                        um[:, None, :].to_broadcast([P, 2, 3 * P]),
                        Mult,
                    )

                    o_ps = ps_o.tile([P, 2, 2, HP1], f32)
                    for s in range(2):
                        nc.tensor.matmul(
                            o_ps[:, s, 0, :],
                            p_sb[:, s, 0:P],
                            v_aug[:, s, 0, :],
                            start=True,
                            stop=True,
                        )
                        nc.tensor.matmul(
                            o_ps[:, s, 1, :],
                            p_sb[:, s, P:T],
                            v_aug[:, s, 0, :],
                            start=True,
                            stop=False,
                        )
                        nc.tensor.matmul(
                            o_ps[:, s, 1, :],
                            p_sb[:, s, T : 3 * P],
                            v_aug[:, s, 1, :],
                            start=False,
                            stop=True,
                        )

                    rec = rpool.tile([P, 2, 2, 1], f32)
                    nc.vector.reciprocal(rec, o_ps[:, :, :, H : H + 1])
                    nc.vector.tensor_tensor(
                        o_sb[:, s0 : s0 + 2, :, :],
                        o_ps[:, :, :, 0:H],
                        rec.to_broadcast([P, 2, 2, H]),
                        Mult,
                    )

                nc.sync.dma_start(
                    out_d[b0 : b0 + Q].rearrange("s (j p) h -> p s j h", p=P),
                    o_sb,
                )

    nc.compile()
    return nc


def _prep_inputs(x, Wq, Wk, Wv, bpc):
    bf = ml_dtypes.bfloat16
    nb = NCORES * bpc
    x = np.asarray(x, dtype=np.float32)[:nb]
    # [b, t, e] -> [b, p, c, t] with e = c*128 + p
    xt = np.ascontiguousarray(
        x.reshape(nb, T, EC, P).transpose(0, 3, 2, 1)
    ).astype(bf)
    wqk = np.concatenate(
        [np.asarray(Wq, np.float32), np.asarray(Wk, np.float32)], axis=1
    )  # [E, 128]
    wqk = np.ascontiguousarray(wqk.reshape(EC, P, P).transpose(1, 0, 2)).astype(bf)
    wv = np.ascontiguousarray(
        np.asarray(Wv, np.float32).reshape(EC, P, H).transpose(1, 0, 2)
    ).astype(bf)
    tril01 = (np.arange(P)[:, None] <= np.arange(P)[None, :]).astype(np.float32)
    um = np.concatenate([tril01, np.ones((P, P), np.float32), tril01], axis=1).astype(
        bf
    )
    iden = np.eye(HP1, dtype=np.float32).astype(bf)
    per_core = []
    for c in range(NCORES):
        per_core.append(
            {
                "xt": xt[c * bpc : (c + 1) * bpc],
                "wqk": wqk,
                "wv": wv,
                "um": um,
                "iden": iden,
            }
        )
    return per_core


def kernel(x, Wq, Wk, Wv, _trace=False, _bpc=BPC):
    """Full inputs in, full output out. Shards batch dim over 8 NeuronCores."""
    from concourse import bass_utils

    if _trace:
        _install_ntff_hook()

    key = ("prog", _bpc)
    if key not in _cache:
        _cache[key] = _build_program(_bpc)
    nc = _cache[key]

    in_maps = _prep_inputs(x, Wq, Wk, Wv, _bpc)
    res = bass_utils.run_bass_kernel_spmd(
        nc, in_maps, core_ids=list(range(NCORES)), trace=_trace
    )
    _cache["last_result"] = res
    out = np.concatenate([r["out"] for r in res.results], axis=0)
    return out.astype(np.float32)
